# revision 7
# baseline (speedup 1.0000x reference)
"""Multi-head attention (B=2, T=2048, D=2048, H=16) on 8 TRN2 NeuronCores.

Tensor-parallel over heads: each core computes 2 heads (dl=256 of D) of the
Q/K/V projections, its heads' attention, and a partial output projection
(columns of Wo). Host sums the 8 partial outputs (the "all-reduce").

Per-core dataflow (bf16 compute, f32 PSUM accumulation):
  QT_h = (Wq_h/sqrt(dh)) @ q^T        [dh=128, BT=4096]  (transposed layout)
  KT_h = Wk_h @ k^T                   [dh, BT]
  V    = v @ Wv_i^T                   [BT, 256]           (natural layout)
  scoresT = KT_h-chunk.T @ QT_h       [k-tile 128, q 512] per (b, h)
  attnT = exp(scoresT) * maskT        (mask multiplicative {0,1}; no max
                                       subtraction needed: |scores| < ~8)
  denom = ones.T @ attnT              (PE column-sum over k, f32 PSUM)
  O^T_h = (V_h-chunk.T @ attnT) * (1/denom)
  partial = O^T.T @ Wo_i^T            [BT, D] -> host sum over cores

Attention groups are software-pipelined: the PE stream is
scores(g+1) | denom(g) attnV(g), so exp(g+1) on ScalarE overlaps PE work
of group g. All DMA sources are host-rearranged to be contiguous.
"""
import sys

if "/opt/trn_rl_repo" not in sys.path:
    sys.path.insert(0, "/opt/trn_rl_repo")

import numpy as np
import ml_dtypes

import concourse.bacc as bacc
import concourse.tile as tile
import concourse.mybir as mybir
from concourse import bass_utils

BF16 = ml_dtypes.bfloat16
FP32 = mybir.dt.float32
BF = mybir.dt.bfloat16

B, T, D, H = 2, 2048, 2048, 16
DH = 128
N_CORES = 8
HL = H // N_CORES          # heads per core = 2
DL = HL * DH               # local out dim = 256
BT = B * T                 # 4096
KC = D // 128              # 16 contraction chunks
NBT = BT // 512            # 8 global bt chunks
NQ = T // 512              # 4 q chunks per batch
NKT = T // 128             # 16 k tiles per batch
ND = D // 512              # 4 D chunks

_CACHE = {}


def _build():
    nc = bacc.Bacc("TRN2", target_bir_lowering=False, debug=False,
                   num_devices=N_CORES)
    # chunk-major transposed activations: [c, p, kc*512+j] = x^T[kc*128+p, c*512+j]
    qTc = nc.dram_tensor("qTc", [NBT, 128, KC * 512], BF, kind="ExternalInput").ap()
    kTc = nc.dram_tensor("kTc", [NBT, 128, KC * 512], BF, kind="ExternalInput").ap()
    vTc = nc.dram_tensor("vTc", [NBT, 128, KC * 512], BF, kind="ExternalInput").ap()
    wq = nc.dram_tensor("wq", [128, KC * DL], BF, kind="ExternalInput").ap()
    wk = nc.dram_tensor("wk", [128, KC * DL], BF, kind="ExternalInput").ap()
    wv = nc.dram_tensor("wv", [128, KC * DL], BF, kind="ExternalInput").ap()
    wo = nc.dram_tensor("wo", [128, HL * D], BF, kind="ExternalInput").ap()
    # tiled multiplicative mask: [b, qc, kt, p, j]
    maskt = nc.dram_tensor("maskt", [B, NQ, NKT, 128, 512], BF,
                           kind="ExternalInput").ap()
    out = nc.dram_tensor("out", [BT, D], BF, kind="ExternalOutput").ap()

    EXP = mybir.ActivationFunctionType.Exp
    MUL = mybir.AluOpType.mult

    with tile.TileContext(nc) as tc:
        with tc.tile_pool(name="wpool", bufs=1) as wpool, \
             tc.tile_pool(name="persist", bufs=1) as ppool, \
             tc.tile_pool(name="stream", bufs=5) as spool, \
             tc.tile_pool(name="mask", bufs=16) as mpool, \
             tc.tile_pool(name="attn", bufs=16) as apool, \
             tc.tile_pool(name="small", bufs=2) as rpool, \
             tc.tile_pool(name="ostage", bufs=3) as opool, \
             tc.tile_pool(name="psbig", bufs=3, space="PSUM") as psbig, \
             tc.tile_pool(name="psacc", bufs=1, space="PSUM") as psacc, \
             tc.tile_pool(name="psden", bufs=1, space="PSUM") as psden:

            # ---- weights + constants ----
            wq_sb = wpool.tile([128, KC * DL], BF, tag="wq")
            wk_sb = wpool.tile([128, KC * DL], BF, tag="wk")
            wv_sb = wpool.tile([128, KC * DL], BF, tag="wv")
            wo_sb = wpool.tile([128, HL * D], BF, tag="wo")
            nc.sync.dma_start(wq_sb[:], wq[:])
            ones = wpool.tile([128, 1], BF, tag="ones")
            nc.vector.memset(ones[:], 1.0)

            # ---- persistent activations (fine-grained for overlap) ----
            QT = [[ppool.tile([128, 512], BF, tag=f"QT{h}_{c}", name=f"QT{h}_{c}")
                   for c in range(NBT)] for h in range(HL)]
            KT = [[ppool.tile([128, T], BF, tag=f"KT{h}_{b}", name=f"KT{h}_{b}")
                   for b in range(B)] for h in range(HL)]
            OT = [[ppool.tile([128, 512], BF, tag=f"OT{h}_{c}", name=f"OT{h}_{c}")
                   for c in range(NBT)] for h in range(HL)]
            V = [ppool.tile([128, DL], BF, tag=f"V{t}", name=f"V{t}")
                 for t in range(BT // 128)]

            def emit_proj(b):
                for cc in range(NQ):
                    c = b * NQ + cc
                    halves = {}

                    def load(nm, srct):
                        for hf in range(2):
                            ch = spool.tile([128, KC // 2, 512], BF,
                                            tag="pin", name=f"pin_{nm}{hf}")
                            nc.sync.dma_start(
                                ch[:],
                                srct[c, :, hf * (KC // 2) * 512:
                                     (hf + 1) * (KC // 2) * 512].rearrange(
                                    "p (kc j) -> p kc j", j=512))
                            halves[nm, hf] = ch

                    def chx(nm, kc):
                        return halves[nm, kc // (KC // 2)][:, kc % (KC // 2), :]

                    load("q", qTc)
                    if b == 0 and cc == 0:
                        nc.sync.dma_start(wk_sb[:], wk[:])
                    load("k", kTc)
                    for m in range(HL):
                        ps = psbig.tile([128, 1024], FP32, tag="mm")
                        for kc in range(KC):
                            nc.tensor.matmul(
                                ps[:, :512],
                                wq_sb[:, kc * DL + m * 128:kc * DL + (m + 1) * 128],
                                chx("q", kc),
                                start=kc == 0, stop=kc == KC - 1)
                        nc.vector.tensor_copy(QT[m][c][:], ps[:, :512])
                        ps2 = psbig.tile([128, 1024], FP32, tag="mm")
                        for kc in range(KC):
                            nc.tensor.matmul(
                                ps2[:, :512],
                                wk_sb[:, kc * DL + m * 128:kc * DL + (m + 1) * 128],
                                chx("k", kc),
                                start=kc == 0, stop=kc == KC - 1)
                        nc.vector.tensor_copy(
                            KT[m][b][:, cc * 512:(cc + 1) * 512], ps2[:, :512])
                    if b == 0 and cc == 0:
                        nc.sync.dma_start(wv_sb[:], wv[:])
                    load("v", vTc)
                    for sub in range(4):
                        t = c * 4 + sub
                        psv = psbig.tile([128, 1024], FP32, tag="mm", name="psv")
                        for kc in range(KC):
                            nc.tensor.matmul(
                                psv[:, :DL],
                                chx("v", kc)[:, sub * 128:(sub + 1) * 128],
                                wv_sb[:, kc * DL:(kc + 1) * DL],
                                start=kc == 0, stop=kc == KC - 1)
                        nc.vector.tensor_copy(V[t][:], psv[:, :DL])

            def emit_attention(b):
                mtiles = {}

                def load_masks(qc):
                    tiles = []
                    for kt in range(NKT):
                        mt = mpool.tile([128, 512], BF, tag="mask")
                        nc.sync.dma_start(mt[:], maskt[b, qc, kt])
                        tiles.append(mt)
                    mtiles[qc] = tiles

                NKP = NKT // 2

                def emit_group(g, prev):
                    """Interleave scores(g) with denom+attnV MMs of prev."""
                    if g is not None:
                        qc, h = g
                        c = b * NQ + qc
                    if prev is not None:
                        (pqc, ph), patl = prev
                        pc = b * NQ + pqc
                        ps_d = psden.tile([1, 512], FP32, tag="den")
                        ps_o = psacc.tile([128, 512], FP32, tag="acc")
                    atiles = []
                    for kp in range(NKP):
                        if g is not None:
                            ps_s = psbig.tile([128, 1024], FP32, tag="mm")
                            for i in range(2):
                                kt = kp * 2 + i
                                nc.tensor.matmul(
                                    ps_s[:, i * 512:(i + 1) * 512],
                                    KT[h][b][:, kt * 128:(kt + 1) * 128],
                                    QT[h][c][:],
                                    start=True, stop=True)
                            at = apool.tile([128, 1024], BF, tag="attn")
                            nc.scalar.activation(at[:], ps_s[:], EXP)
                            for i in range(2):
                                kt = kp * 2 + i
                                nc.vector.tensor_tensor(
                                    at[:, i * 512:(i + 1) * 512],
                                    at[:, i * 512:(i + 1) * 512],
                                    mtiles[qc][kt][:], op=MUL)
                            atiles.append(at)
                        if prev is not None:
                            for i in range(2):
                                kt = kp * 2 + i
                                nc.tensor.matmul(
                                    ps_d[:], ones[:],
                                    patl[kp][:, i * 512:(i + 1) * 512],
                                    start=(kp == 0 and i == 0),
                                    stop=(kp == NKP - 1 and i == 1),
                                    skip_group_check=True)
                                nc.tensor.matmul(
                                    ps_o[:],
                                    V[b * NKT + kt][:, ph * 128:(ph + 1) * 128],
                                    patl[kp][:, i * 512:(i + 1) * 512],
                                    start=(kp == 0 and i == 0),
                                    stop=(kp == NKP - 1 and i == 1),
                                    skip_group_check=True)
                    if prev is not None:
                        rec = rpool.tile([1, 512], FP32, tag="rec")
                        nc.vector.reciprocal_approx_fast(rec[:], ps_d[:])
                        rbc = rpool.tile([128, 512], FP32, tag="rbc")
                        nc.gpsimd.partition_broadcast(rbc[:], rec[:])
                        nc.vector.scalar_tensor_tensor(
                            OT[ph][pc][:], ps_o[:], 1.0, rbc[:],
                            op0=MUL, op1=MUL)
                    return atiles

                groups = [(qc, h) for qc in range(NQ) for h in range(HL)]
                prev = None
                for g in groups:
                    if g[1] == 0:
                        load_masks(g[0])
                    atl = emit_group(g, prev)
                    prev = (g, atl)
                emit_group(None, prev)

            def emit_outproj(b):
                if b == 0:
                    nc.sync.dma_start(wo_sb[:], wo[:])
                for qc in range(NQ):
                    c = b * NQ + qc
                    for sub in range(4):
                        t = c * 4 + sub
                        stage = opool.tile([128, D], BF, tag="ostage")
                        for dp in range(2):
                            ps = psbig.tile([128, 1024], FP32, tag="mm")
                            for i in range(2):
                                dc = dp * 2 + i
                                for h in range(HL):
                                    nc.tensor.matmul(
                                        ps[:, i * 512:(i + 1) * 512],
                                        OT[h][c][:, sub * 128:(sub + 1) * 128],
                                        wo_sb[:, h * D + dc * 512:
                                              h * D + (dc + 1) * 512],
                                        start=h == 0, stop=h == HL - 1)
                            if dp == 0:
                                nc.scalar.copy(stage[:, :1024], ps[:])
                            else:
                                nc.vector.tensor_copy(stage[:, 1024:], ps[:])
                        nc.sync.dma_start(out[t * 128:(t + 1) * 128, :], stage[:])

            for b in range(B):
                emit_proj(b)
                emit_attention(b)
                emit_outproj(b)

    nc.compile()
    return nc


def get_nc():
    if "nc" not in _CACHE:
        _CACHE["nc"] = _build()
    return _CACHE["nc"]


def make_in_maps(q, k, v, Wq, Wk, Wv, Wo, attn_mask, key_padding_mask):
    scale = np.float32(1.0 / np.sqrt(np.float32(DH)))

    def prep_x(x):  # [BT, D] -> chunk-major [NBT, 128, KC*512] of x^T
        xT = x.reshape(BT, D).T.astype(BF16)            # [D, BT]
        a = xT.reshape(KC, 128, NBT, 512).transpose(2, 1, 0, 3)
        return np.ascontiguousarray(a.reshape(NBT, 128, KC * 512))

    qTc = prep_x(q)
    kTc = prep_x(k)
    vTc = prep_x(v)
    # multiplicative transposed mask, tiled [B, NQ, NKT, 128, 512]
    m = ~(key_padding_mask[:, :, None] | attn_mask.T[None, :, :])  # [B,TK,TQ]
    mt = m.astype(BF16).reshape(B, NKT, 128, NQ, 512).transpose(0, 3, 1, 2, 4)
    maskt = np.ascontiguousarray(mt)

    def prep_w(wT):  # [D, DL] -> [128, KC*DL]
        return np.ascontiguousarray(
            wT.reshape(KC, 128, DL).transpose(1, 0, 2).reshape(128, KC * DL)
            .astype(BF16))

    in_maps = []
    for i in range(N_CORES):
        rows = slice(i * DL, (i + 1) * DL)
        wq_i = prep_w(Wq[rows, :].T * scale)
        wk_i = prep_w(Wk[rows, :].T)
        wv_i = prep_w(Wv[rows, :].T)
        woT = Wo[:, rows].T  # [DL, D]
        wo_i = np.ascontiguousarray(
            woT.reshape(HL, 128, D).transpose(1, 0, 2).reshape(128, HL * D)
            .astype(BF16))
        in_maps.append({
            "qTc": qTc, "kTc": kTc, "vTc": vTc,
            "wq": wq_i, "wk": wk_i, "wv": wv_i, "wo": wo_i,
            "maskt": maskt,
        })
    return in_maps


def postprocess(results):
    acc = np.zeros((BT, D), np.float32)
    for r in results:
        acc += r["out"].astype(np.float32)
    return acc.reshape(B, T, D)


def kernel(**inputs):
    inputs = {k: np.asarray(v) for k, v in inputs.items()}
    nc = get_nc()
    in_maps = make_in_maps(**inputs)
    res = bass_utils.run_bass_kernel_spmd(
        nc, in_maps, core_ids=list(range(N_CORES)))
    return postprocess(res.results)


# revision 8
# speedup vs baseline: 1.0933x; 1.0933x over previous
"""Multi-head attention (B=2, T=2048, D=2048, H=16) on 8 TRN2 NeuronCores.

Tensor-parallel over heads: each core computes 2 heads (dl=256 of D) of the
Q/K/V projections, its heads' attention, and a partial output projection
(columns of Wo). Host sums the 8 partial outputs (the "all-reduce").

Per-core dataflow (bf16 compute, f32 PSUM accumulation):
  QT_h = (Wq_h/sqrt(dh)) @ q^T        [dh=128, BT=4096]  (transposed layout)
  KT_h = Wk_h @ k^T                   [dh, BT]
  V    = v @ Wv_i^T                   [BT, 256]           (natural layout)
  scoresT = KT_h-chunk.T @ QT_h       [k-tile 128, q 512] per (b, h)
  attnT = exp(scoresT) * maskT        (mask multiplicative {0,1}; no max
                                       subtraction needed: |scores| < ~8)
  denom = ones.T @ attnT              (PE column-sum over k, f32 PSUM)
  O^T_h = (V_h-chunk.T @ attnT) * (1/denom)
  partial = O^T.T @ Wo_i^T            [BT, D] -> host sum over cores

Attention groups are software-pipelined: the PE stream is
scores(g+1) | denom(g) attnV(g), so exp(g+1) on ScalarE overlaps PE work
of group g. All DMA sources are host-rearranged to be contiguous.
"""
import sys

if "/opt/trn_rl_repo" not in sys.path:
    sys.path.insert(0, "/opt/trn_rl_repo")

import numpy as np
import ml_dtypes

import concourse.bacc as bacc
import concourse.tile as tile
import concourse.mybir as mybir
from concourse import bass_utils

BF16 = ml_dtypes.bfloat16
FP32 = mybir.dt.float32
BF = mybir.dt.bfloat16

B, T, D, H = 2, 2048, 2048, 16
DH = 128
N_CORES = 8
HL = H // N_CORES          # heads per core = 2
DL = HL * DH               # local out dim = 256
BT = B * T                 # 4096
KC = D // 128              # 16 contraction chunks
NBT = BT // 512            # 8 global bt chunks
NQ = T // 512              # 4 q chunks per batch
NKT = T // 128             # 16 k tiles per batch
ND = D // 512              # 4 D chunks

_CACHE = {}


def _build():
    nc = bacc.Bacc("TRN2", target_bir_lowering=False, debug=False,
                   num_devices=N_CORES)
    # chunk-major transposed activations: [c, p, kc*512+j] = x^T[kc*128+p, c*512+j]
    qTc = nc.dram_tensor("qTc", [NBT, 128, KC * 512], BF, kind="ExternalInput").ap()
    kTc = nc.dram_tensor("kTc", [NBT, 128, KC * 512], BF, kind="ExternalInput").ap()
    vTc = nc.dram_tensor("vTc", [NBT, 128, KC * 512], BF, kind="ExternalInput").ap()
    wq = nc.dram_tensor("wq", [128, KC * DL], BF, kind="ExternalInput").ap()
    wk = nc.dram_tensor("wk", [128, KC * DL], BF, kind="ExternalInput").ap()
    wv = nc.dram_tensor("wv", [128, KC * DL], BF, kind="ExternalInput").ap()
    wo = nc.dram_tensor("wo", [128, HL * D], BF, kind="ExternalInput").ap()
    # tiled multiplicative mask: [b, qc, kt, p, j]
    maskt = nc.dram_tensor("maskt", [B, NQ, NKT, 128, 512], BF,
                           kind="ExternalInput").ap()
    out = nc.dram_tensor("out", [BT, D], BF, kind="ExternalOutput").ap()

    EXP = mybir.ActivationFunctionType.Exp
    MUL = mybir.AluOpType.mult

    with tile.TileContext(nc) as tc:
        with tc.tile_pool(name="wpool", bufs=1) as wpool, \
             tc.tile_pool(name="persist", bufs=1) as ppool, \
             tc.tile_pool(name="stream", bufs=5) as spool, \
             tc.tile_pool(name="mask", bufs=16) as mpool, \
             tc.tile_pool(name="attn", bufs=16) as apool, \
             tc.tile_pool(name="small", bufs=2) as rpool, \
             tc.tile_pool(name="ostage", bufs=3) as opool, \
             tc.tile_pool(name="psbig", bufs=3, space="PSUM") as psbig, \
             tc.tile_pool(name="psacc", bufs=1, space="PSUM") as psacc, \
             tc.tile_pool(name="psden", bufs=1, space="PSUM") as psden:

            # ---- weights + constants ----
            wq_sb = wpool.tile([128, KC * DL], BF, tag="wq")
            wk_sb = wpool.tile([128, KC * DL], BF, tag="wk")
            wv_sb = wpool.tile([128, KC * DL], BF, tag="wv")
            wo_sb = wpool.tile([128, HL * D], BF, tag="wo")
            nc.sync.dma_start(wq_sb[:], wq[:])
            ones = wpool.tile([128, 1], BF, tag="ones")
            nc.vector.memset(ones[:], 1.0)

            # ---- persistent activations (fine-grained for overlap) ----
            QT = [[ppool.tile([128, 512], BF, tag=f"QT{h}_{c}", name=f"QT{h}_{c}")
                   for c in range(NBT)] for h in range(HL)]
            KT = [[ppool.tile([128, T], BF, tag=f"KT{h}_{b}", name=f"KT{h}_{b}")
                   for b in range(B)] for h in range(HL)]
            OT = [[ppool.tile([128, 512], BF, tag=f"OT{h}_{c}", name=f"OT{h}_{c}")
                   for c in range(NBT)] for h in range(HL)]
            V = [ppool.tile([128, DL], BF, tag=f"V{t}", name=f"V{t}")
                 for t in range(BT // 128)]

            def emit_proj(b):
                for cc in range(NQ):
                    c = b * NQ + cc
                    halves = {}

                    def load(nm, srct):
                        for hf in range(2):
                            ch = spool.tile([128, KC // 2, 512], BF,
                                            tag="pin", name=f"pin_{nm}{hf}")
                            nc.sync.dma_start(
                                ch[:],
                                srct[c, :, hf * (KC // 2) * 512:
                                     (hf + 1) * (KC // 2) * 512].rearrange(
                                    "p (kc j) -> p kc j", j=512))
                            halves[nm, hf] = ch

                    def chx(nm, kc):
                        return halves[nm, kc // (KC // 2)][:, kc % (KC // 2), :]

                    load("q", qTc)
                    if b == 0 and cc == 0:
                        nc.sync.dma_start(wk_sb[:], wk[:])
                    load("k", kTc)
                    for m in range(HL):
                        ps = psbig.tile([128, 1024], FP32, tag="mm")
                        for kc in range(KC):
                            nc.tensor.matmul(
                                ps[:, :512],
                                wq_sb[:, kc * DL + m * 128:kc * DL + (m + 1) * 128],
                                chx("q", kc),
                                start=kc == 0, stop=kc == KC - 1)
                        nc.vector.tensor_copy(QT[m][c][:], ps[:, :512])
                        ps2 = psbig.tile([128, 1024], FP32, tag="mm")
                        for kc in range(KC):
                            nc.tensor.matmul(
                                ps2[:, :512],
                                wk_sb[:, kc * DL + m * 128:kc * DL + (m + 1) * 128],
                                chx("k", kc),
                                start=kc == 0, stop=kc == KC - 1)
                        nc.vector.tensor_copy(
                            KT[m][b][:, cc * 512:(cc + 1) * 512], ps2[:, :512])
                    if b == 0 and cc == 0:
                        nc.sync.dma_start(wv_sb[:], wv[:])
                    load("v", vTc)
                    for sub in range(4):
                        t = c * 4 + sub
                        psv = psbig.tile([128, 1024], FP32, tag="mm", name="psv")
                        for kc in range(KC):
                            nc.tensor.matmul(
                                psv[:, :DL],
                                chx("v", kc)[:, sub * 128:(sub + 1) * 128],
                                wv_sb[:, kc * DL:(kc + 1) * DL],
                                start=kc == 0, stop=kc == KC - 1)
                        nc.vector.tensor_copy(V[t][:], psv[:, :DL])

            def emit_attention(b):
                mtiles = {}

                def load_masks(qc):
                    tiles = []
                    for kt in range(NKT):
                        mt = mpool.tile([128, 512], BF, tag="mask")
                        nc.sync.dma_start(mt[:], maskt[b, qc, kt])
                        tiles.append(mt)
                    mtiles[qc] = tiles

                def emit_scores(g):
                    qc, h = g
                    c = b * NQ + qc
                    atiles = []
                    for kp in range(NKT // 2):
                        ps_s = psbig.tile([128, 1024], FP32, tag="mm")
                        for i in range(2):
                            kt = kp * 2 + i
                            nc.tensor.matmul(
                                ps_s[:, i * 512:(i + 1) * 512],
                                KT[h][b][:, kt * 128:(kt + 1) * 128],
                                QT[h][c][:],
                                start=True, stop=True)
                        at = apool.tile([128, 1024], BF, tag="attn")
                        nc.scalar.activation(at[:], ps_s[:], EXP)
                        for i in range(2):
                            kt = kp * 2 + i
                            nc.vector.tensor_tensor(
                                at[:, i * 512:(i + 1) * 512],
                                at[:, i * 512:(i + 1) * 512],
                                mtiles[qc][kt][:], op=MUL)
                        atiles.append(at)
                    return atiles

                def emit_tail(g, atiles):
                    qc, h = g
                    c = b * NQ + qc
                    ps_d = psden.tile([1, 512], FP32, tag="den")
                    for kp in range(NKT // 2):
                        for i in range(2):
                            nc.tensor.matmul(
                                ps_d[:], ones[:],
                                atiles[kp][:, i * 512:(i + 1) * 512],
                                start=(kp == 0 and i == 0),
                                stop=(kp == NKT // 2 - 1 and i == 1))
                    rec = rpool.tile([1, 512], FP32, tag="rec")
                    nc.vector.reciprocal_approx_fast(rec[:], ps_d[:])
                    rbc = rpool.tile([128, 512], FP32, tag="rbc")
                    nc.gpsimd.partition_broadcast(rbc[:], rec[:])
                    ps_o = psacc.tile([128, 512], FP32, tag="acc")
                    for kp in range(NKT // 2):
                        for i in range(2):
                            kt = kp * 2 + i
                            nc.tensor.matmul(
                                ps_o[:],
                                V[b * NKT + kt][:, h * 128:(h + 1) * 128],
                                atiles[kp][:, i * 512:(i + 1) * 512],
                                start=(kp == 0 and i == 0),
                                stop=(kp == NKT // 2 - 1 and i == 1))
                    nc.vector.scalar_tensor_tensor(
                        OT[h][c][:], ps_o[:], 1.0, rbc[:], op0=MUL, op1=MUL)

                groups = [(qc, h) for qc in range(NQ) for h in range(HL)]
                prev = None
                for g in groups:
                    if g[1] == 0:
                        load_masks(g[0])
                    atl = emit_scores(g)
                    if prev is not None:
                        emit_tail(*prev)
                    prev = (g, atl)
                emit_tail(*prev)

            def emit_outproj(b):
                if b == 0:
                    nc.sync.dma_start(wo_sb[:], wo[:])
                for qc in range(NQ):
                    c = b * NQ + qc
                    for sub in range(4):
                        t = c * 4 + sub
                        stage = opool.tile([128, D], BF, tag="ostage")
                        for dp in range(2):
                            ps = psbig.tile([128, 1024], FP32, tag="mm")
                            for i in range(2):
                                dc = dp * 2 + i
                                for h in range(HL):
                                    nc.tensor.matmul(
                                        ps[:, i * 512:(i + 1) * 512],
                                        OT[h][c][:, sub * 128:(sub + 1) * 128],
                                        wo_sb[:, h * D + dc * 512:
                                              h * D + (dc + 1) * 512],
                                        start=h == 0, stop=h == HL - 1)
                            if dp == 0:
                                nc.scalar.copy(stage[:, :1024], ps[:])
                            else:
                                nc.vector.tensor_copy(stage[:, 1024:], ps[:])
                        nc.sync.dma_start(out[t * 128:(t + 1) * 128, :], stage[:])

            for b in range(B):
                emit_proj(b)
                emit_attention(b)
                emit_outproj(b)

    nc.compile()
    return nc


def get_nc():
    if "nc" not in _CACHE:
        _CACHE["nc"] = _build()
    return _CACHE["nc"]


def make_in_maps(q, k, v, Wq, Wk, Wv, Wo, attn_mask, key_padding_mask):
    scale = np.float32(1.0 / np.sqrt(np.float32(DH)))

    def prep_x(x):  # [BT, D] -> chunk-major [NBT, 128, KC*512] of x^T
        xT = x.reshape(BT, D).T.astype(BF16)            # [D, BT]
        a = xT.reshape(KC, 128, NBT, 512).transpose(2, 1, 0, 3)
        return np.ascontiguousarray(a.reshape(NBT, 128, KC * 512))

    qTc = prep_x(q)
    kTc = prep_x(k)
    vTc = prep_x(v)
    # multiplicative transposed mask, tiled [B, NQ, NKT, 128, 512]
    m = ~(key_padding_mask[:, :, None] | attn_mask.T[None, :, :])  # [B,TK,TQ]
    mt = m.astype(BF16).reshape(B, NKT, 128, NQ, 512).transpose(0, 3, 1, 2, 4)
    maskt = np.ascontiguousarray(mt)

    def prep_w(wT):  # [D, DL] -> [128, KC*DL]
        return np.ascontiguousarray(
            wT.reshape(KC, 128, DL).transpose(1, 0, 2).reshape(128, KC * DL)
            .astype(BF16))

    in_maps = []
    for i in range(N_CORES):
        rows = slice(i * DL, (i + 1) * DL)
        wq_i = prep_w(Wq[rows, :].T * scale)
        wk_i = prep_w(Wk[rows, :].T)
        wv_i = prep_w(Wv[rows, :].T)
        woT = Wo[:, rows].T  # [DL, D]
        wo_i = np.ascontiguousarray(
            woT.reshape(HL, 128, D).transpose(1, 0, 2).reshape(128, HL * D)
            .astype(BF16))
        in_maps.append({
            "qTc": qTc, "kTc": kTc, "vTc": vTc,
            "wq": wq_i, "wk": wk_i, "wv": wv_i, "wo": wo_i,
            "maskt": maskt,
        })
    return in_maps


def postprocess(results):
    acc = np.zeros((BT, D), np.float32)
    for r in results:
        acc += r["out"].astype(np.float32)
    return acc.reshape(B, T, D)


def kernel(**inputs):
    inputs = {k: np.asarray(v) for k, v in inputs.items()}
    nc = get_nc()
    in_maps = make_in_maps(**inputs)
    res = bass_utils.run_bass_kernel_spmd(
        nc, in_maps, core_ids=list(range(N_CORES)))
    return postprocess(res.results)
